# revision 23
# baseline (speedup 1.0000x reference)
"""Trainium2 Bass kernel for nn_Axial_PFCU_Continuous (dense_cnn).

All linear terms ride the PE:
  z = W0 @ x  (bf16; W0 = GAMMA*(Wf~ diag(c0) + diag(cB0)))
      + 16 shift terms (mixer taps at +-4/8/16 on H and W, edge taps at +-1)
        packed as 8 fp8 DoubleRow matmuls, two terms per matmul: the rhs is a
        zero-padded fp8 image and the DoubleRow "t" dimension is given an
        arbitrary 2-D displacement stride, so each matmul contracts two
        different shifted windows of the same buffer (cost: 0.5 cyc/pixel
        per matmul, contraction-independent).
  pre = PReLU(z/16 + bz) on Act; coord-attention stats on DVE/Pool.

The image is processed in two column halves (left cols 0:64, then right),
so the left half's aw gating + output DMA overlap the right half's compute.

Sharding: pure data-parallel, 1 of 8 batch samples per NeuronCore.
"""
import sys
import math

sys.path.insert(0, '/opt/trn_rl_repo')

import numpy as np
import ml_dtypes
from contextlib import ExitStack

import concourse.bass as bass
import concourse.bacc as bacc
from concourse import mybir, tile
from concourse.bass_utils import run_bass_kernel_spmd
from concourse.ap import AP as APC

f32 = mybir.dt.float32
bf16 = mybir.dt.bfloat16
fp8 = mybir.dt.float8e4
ALU = mybir.AluOpType
AF = mybir.ActivationFunctionType
PM = mybir.MatmulPerfMode
AX = mybir.AxisListType

B, C, H, W = 8, 96, 128, 128
HW = H * W
PAD = 16
HP, WP = H + 2 * PAD, W + 2 * PAD      # padded fp8 image dims (160 x 160)
EPS = 1e-5
GAMMA = 16.0
HB = W // 2                            # half-width column split
BH = 8                                 # rows per block
NB = H // BH                           # 16 blocks per phase
N_CORES = 8
# ah groups (closed after the right-phase block that completes each row range)
GROUPS = {3: (0, 32), 7: (32, 64), 11: (64, 96), 12: (96, 104),
          13: (104, 112), 14: (112, 120), 15: (120, 128)}

_GRAPH_CACHE = {}


# ----------------------------------------------------------------- host folds
def _taps(w_taps, r):
    r = max(float(r), 1.0)
    K = w_taps.shape[1]
    d2w = {}
    for i in range(K):
        s = (i - K // 2) * r
        f = math.floor(s)
        frac = s - f
        for d, wt in ((int(f), 1.0 - frac), (int(f) + 1, frac)):
            if wt != 0.0:
                if d not in d2w:
                    d2w[d] = np.zeros(C, np.float64)
                d2w[d] = d2w[d] + wt * np.asarray(w_taps[:, i], np.float64)
    return {d: w for d, w in d2w.items() if abs(d) < H}


def _merge(a, b):
    out = dict(a)
    for d, w in b.items():
        out[d] = out.get(d, np.zeros(C, np.float64)) + w
    return out


class _Pack:
    def __init__(self, rows):
        self.rows = rows
        self.cols = {}
        self.parts = []
        self.pos = 0

    def put(self, name, arr):
        arr = np.asarray(arr, np.float64)
        if arr.ndim == 1:
            arr = arr[:, None]
        pad = np.zeros((self.rows, arr.shape[1]), np.float64)
        pad[:arr.shape[0], :] = arr
        self.cols[name] = (self.pos, arr.shape[1])
        self.parts.append(pad)
        self.pos += arr.shape[1]

    def done(self, dt):
        return np.concatenate(self.parts, axis=1).astype(dt)


def _fold(inp):
    g = lambda k: np.asarray(inp[k], np.float64)
    hA = _merge(_taps(g('wh_m'), float(np.asarray(inp['r_m']))),
                _taps(g('wh_l'), float(np.asarray(inp['r_l']))))
    wA = _merge(_taps(g('ww_m'), float(np.asarray(inp['r_m']))),
                _taps(g('ww_l'), float(np.asarray(inp['r_l']))))
    hA[0] = hA.get(0, np.zeros(C)) + 2.0    # identity terms of m+l
    wA.setdefault(0, np.zeros(C))
    c0 = hA[0] + wA[0]

    sf = g('bnf_g') / np.sqrt(g('bnf_v') + EPS)
    wf = g('w_fuse') * sf[:, None]            # (Cout, Cin) BN-folded
    bf_ = g('bnf_b') - g('bnf_m') * sf

    ds = g('dg_g') / np.sqrt(g('dg_v') + EPS)
    db = g('dg_b') - g('dg_m') * ds
    dg_wh, dg_ww = g('dg_wh'), g('dg_ww')
    ehm1, eh0, ehp1 = ds * dg_wh[:, 0], ds * (dg_wh[:, 1] + 1.0), ds * dg_wh[:, 2]
    ewm1, ew0, ewp1 = ds * dg_ww[:, 0], ds * dg_ww[:, 1], ds * dg_ww[:, 2]
    cB0 = eh0 + ew0
    bz = bf_ + db

    cs = g('ca_g') / np.sqrt(g('ca_v') + EPS)
    cb = g('ca_b') - g('ca_m') * cs

    # fp8 PE terms: (dr, dc, (Cout, Cin) matrix), all GAMMA-scaled
    terms = []
    for d in sorted(hA):
        if d != 0:
            assert abs(d) <= PAD, d
            terms.append((d, 0, GAMMA * wf * hA[d][None, :]))
    for d in sorted(wA):
        if d != 0:
            assert abs(d) <= PAD, d
            terms.append((0, d, GAMMA * wf * wA[d][None, :]))
    terms.append((-1, 0, np.diag(GAMMA * ehm1)))
    terms.append((1, 0, np.diag(GAMMA * ehp1)))
    terms.append((0, -1, np.diag(GAMMA * ewm1)))
    terms.append((0, 1, np.diag(GAMMA * ewp1)))
    if len(terms) % 2:
        terms.append((0, 0, np.zeros((C, C))))
    terms.sort(key=lambda t: t[0] * WP + t[1])

    pairs = []
    pkq = _Pack(C)
    for i in range(0, len(terms), 2):
        r0_, c0_, A0 = terms[i]
        r1_, c1_, A1 = terms[i + 1]
        s_t = (r1_ - r0_) * WP + (c1_ - c0_)
        assert s_t > 0, (terms[i][:2], terms[i + 1][:2])
        pairs.append(((r0_, c0_), s_t))
        pkq.put(f'P{i // 2}', np.concatenate([A0.T, A1.T], axis=1))
    constq = pkq.done(ml_dtypes.float8_e4m3)

    pkb = _Pack(C)
    W0 = GAMMA * (wf * c0[None, :] + np.diag(cB0))
    pkb.put('W0T', W0.T)
    pkb.put('caw1_tb', (g('ca_w1') / float(W)).T)     # (C, 8)
    pkb.put('caww_tb', g('ca_ww').T)                  # (8, C)
    constb = pkb.done(ml_dtypes.bfloat16)

    pkf = _Pack(C)
    pkf.put('bz', bz)
    pkf.put('act_a', g('act_a'))
    pkf.put('zero', np.zeros(C))
    pkf.put('caw1_t', (g('ca_w1') / float(W)).T)      # (C, 8) f32
    pkf.put('cas', cs)
    pkf.put('cab', cb)
    pkf.put('caa', g('ca_a'))
    pkf.put('cawh_t', g('ca_wh').T)                   # (8, C)
    consts = pkf.done(np.float32)

    key = (tuple(pairs), consts.shape[1], constb.shape[1], constq.shape[1])
    return consts, pkf.cols, constb, pkb.cols, constq, pkq.cols, pairs, key


# -------------------------------------------------------------- graph builder
def _build(pairs, colf, colb, colq, ckf, ckb, ckq):
    nc = bacc.Bacc()
    xb_p = nc.declare_dram_parameter("xb", (C, HW), bf16, isOutput=False)
    xp_p = nc.declare_dram_parameter("xpad", (C, HP * WP), fp8, isOutput=False)
    cf_p = nc.declare_dram_parameter("consts", (C, ckf), f32, isOutput=False)
    cb_p = nc.declare_dram_parameter("constb", (C, ckb), bf16, isOutput=False)
    cq_p = nc.declare_dram_parameter("constq", (C, ckq), fp8, isOutput=False)
    # out laid out as [C, side(2), H, HB]; host concatenates the halves
    o_p = nc.declare_dram_parameter("out", (C, HW), bf16, isOutput=True)

    with tile.TileContext(nc) as tc, ExitStack() as ctx:
        big = ctx.enter_context(tc.tile_pool(name="big", bufs=1))
        f1p = ctx.enter_context(tc.tile_pool(name="f1p", bufs=4))
        f2p = ctx.enter_context(tc.tile_pool(name="f2p", bufs=4))
        awf = ctx.enter_context(tc.tile_pool(name="awf", bufs=2))
        y2p = ctx.enter_context(tc.tile_pool(name="y2p", bufs=2))
        psq = ctx.enter_context(tc.tile_pool(name="psq", bufs=5, space="PSUM"))
        psw = ctx.enter_context(tc.tile_pool(name="psw", bufs=1, space="PSUM"))
        pss = ctx.enter_context(tc.tile_pool(name="pss", bufs=2, space="PSUM"))

        cst = big.tile([C, ckf], f32, tag="cst")
        cbt = big.tile([C, ckb], bf16, tag="cbt")
        cqt = big.tile([C, ckq], fp8, tag="cqt")

        def cc(name):
            p0, _ = colf[name]
            return cst[:, p0:p0 + 1]

        def crf(name, rows=C):
            p0, n = colf[name]
            return cst[0:rows, p0:p0 + n]

        def cbr(name, rows=C):
            p0, n = colb[name]
            return cbt[0:rows, p0:p0 + n]

        def cq(i):
            p0, n = colq[f'P{i}']
            return cqt[0:C, p0:p0 + n].rearrange("p (t m) -> p t m", t=2)

        xb_sb = big.tile([C, HW], bf16, tag="xb")
        xp_sb = big.tile([C, HP * WP], fp8, tag="xpad")
        preL = big.tile([C, H * HB], bf16, tag="preL")
        preR = big.tile([C, H * HB], bf16, tag="preR")
        scr = big.tile([C, 512], bf16, tag="scr")
        xwacL = big.tile([C, BH * HB], bf16, tag="xwacL")
        xwacR = big.tile([C, BH * HB], bf16, tag="xwacR")
        yinL = big.tile([C, H], f32, tag="yinL")
        yinR = big.tile([C, H], f32, tag="yinR")
        ying = big.tile([C, H], f32, tag="ying")
        ah = big.tile([C, H], bf16, tag="ah")
        awL = big.tile([C, HB], bf16, tag="awL")
        awR = big.tile([C, HB], bf16, tag="awR")
        xwL = big.tile([C, HB], bf16, tag="xwL")
        xwR = big.tile([C, HB], bf16, tag="xwR")

        xb3 = xb_sb[:].rearrange("p (h w) -> p h w", w=W)
        preL3 = preL[:].rearrange("p (h w) -> p h w", w=HB)
        preR3 = preR[:].rearrange("p (h w) -> p h w", w=HB)
        xwacL3 = xwacL[:].rearrange("p (h w) -> p h w", w=HB)
        xwacR3 = xwacR[:].rearrange("p (h w) -> p h w", w=HB)
        o3 = o_p[:].rearrange("p (s h w) -> p s h w", s=2, w=HB)
        xp_t = xp_sb[:].tensor
        zcol = cc('zero')

        def xpdma(eng, a, b):
            eng.dma_start(xp_sb[:, a * WP:b * WP], xp_p[:, a * WP:b * WP])

        def xbdma(eng, a, b):
            eng.dma_start(xb_sb[:, a * W:b * W], xb_p[:, a * W:b * W])

        # ---- input DMA schedule (first-needed first, 3 queues) ----
        xbdma(nc.scalar, 0, 8)
        nc.scalar.dma_start(cbt[:], cb_p[:])
        nc.scalar.dma_start(cqt[:], cq_p[:])
        nc.scalar.dma_start(cst[:], cf_p[:])
        xbdma(nc.scalar, 8, 16)
        xbdma(nc.scalar, 16, 32)
        xpdma(nc.sync, 0, 22)
        xpdma(nc.sync, 44, 66)
        xbdma(nc.sync, 32, 56)
        xbdma(nc.sync, 56, 88)
        xbdma(nc.sync, 88, 128)
        nc.gpsimd.memset(scr[:], 0.0)
        xpdma(nc.gpsimd, 22, 44)
        xpdma(nc.gpsimd, 66, 98)
        xpdma(nc.gpsimd, 98, 130)
        xpdma(nc.gpsimd, 130, 160)

        # ---- PE p-state warmup on zeroed scratch ----
        for i in range(4):
            pw = psw.tile([C, 512], f32, tag="warm")
            nc.tensor.matmul(pw[:], scr[:, 0:96], scr[:], start=True, stop=True)

        def qrhs(r0, cside, pair):
            (dr, dc), s_t = pair
            off = (r0 + PAD + dr) * WP + (PAD + dc + cside)
            return APC(xp_t, off, [[HP * WP, C], [s_t, 2], [WP, BH], [1, HB]])

        def do_block(side, b, pre3, yinX, xwac3, first=False, late=False):
            r0, r1 = b * BH, (b + 1) * BH
            cs_ = side * HB
            pk = psq.tile([C, BH, HB], f32, tag="pk")
            nc.tensor.matmul(pk[:], cbr('W0T'), xb3[:, r0:r1, cs_:cs_ + HB],
                             start=True, stop=False)
            for i, pr in enumerate(pairs):
                nc.tensor.matmul(pk[:], cq(i), qrhs(r0, cs_, pr),
                                 start=False, stop=(i == len(pairs) - 1),
                                 perf_mode=PM.DoubleRow)
            nc.scalar.activation(pre3[:, r0:r1, :], pk[:], AF.Prelu,
                                 bias=cc('bz'), scale=1.0 / GAMMA,
                                 alpha=cc('act_a'))
            if first:
                nc.vector.tensor_copy(xwac3, pre3[:, r0:r1, :])
            else:
                xw_eng = nc.gpsimd if late else nc.vector
                xw_eng.tensor_tensor(xwac3, xwac3, pre3[:, r0:r1, :],
                                     op=ALU.add)
            f1 = f1p.tile([C, BH * (HB // 2)], bf16, tag="f1")
            f13 = f1[:].rearrange("p (h w) -> p h w", w=HB // 2)
            nc.gpsimd.tensor_tensor(f13, pre3[:, r0:r1, 0:HB // 2],
                                    pre3[:, r0:r1, HB // 2:HB], op=ALU.add)
            f2 = f2p.tile([C, BH * (HB // 4)], bf16, tag="f2")
            f23 = f2[:].rearrange("p (h w) -> p h w", w=HB // 4)
            nc.gpsimd.tensor_tensor(f23, f13[:, :, 0:HB // 4],
                                    f13[:, :, HB // 4:HB // 2], op=ALU.add)
            nc.vector.tensor_reduce(yinX[:, r0:r1], f23, axis=AX.X, op=ALU.add)

        def aw_chain(xwac3, xwX, awX, eng=None):
            eng = eng or nc.vector
            t1 = awf.tile([C, 4 * HB], bf16, tag="awt1")
            t13 = t1[:].rearrange("p (h w) -> p h w", w=HB)
            eng.tensor_tensor(t13, xwac3[:, 0:4, :], xwac3[:, 4:8, :],
                              op=ALU.add)
            t2 = awf.tile([C, 2 * HB], bf16, tag="awt2")
            t23 = t2[:].rearrange("p (h w) -> p h w", w=HB)
            eng.tensor_tensor(t23, t13[:, 0:2, :], t13[:, 2:4, :],
                              op=ALU.add)
            eng.tensor_tensor(xwX[:], t2[:, 0:HB], t2[:, HB:2 * HB],
                              op=ALU.add)
            y1wt = pss.tile([C, 512], f32, tag="small")
            y1w = y1wt[0:8, 0:HB]
            nc.tensor.matmul(y1w, cbr('caw1_tb'), xwX[:],
                             start=True, stop=True)
            y2w = y2p.tile([8, HB], bf16, tag="y2w")
            nc.scalar.activation(y2w[:], y1w, AF.Prelu,
                                 bias=cc('cab')[0:8, :], scale=cc('cas')[0:8, :],
                                 alpha=cc('caa')[0:8, :])
            awpt = pss.tile([C, 512], f32, tag="small")
            awp = awpt[0:C, 0:HB]
            nc.tensor.matmul(awp, cbr('caww_tb', rows=8), y2w[:],
                             start=True, stop=True)
            nc.scalar.activation(awX[:], awp, AF.Sigmoid, bias=zcol,
                                 scale=1.0)

        def ca_group(g0, g1):
            nc.vector.tensor_tensor(ying[:, g0:g1], yinL[:, g0:g1],
                                    yinR[:, g0:g1], op=ALU.add)
            y1t = pss.tile([C, 512], f32, tag="small")
            y1 = y1t[0:8, 0:g1 - g0]
            nc.tensor.matmul(y1, crf('caw1_t'), ying[:, g0:g1],
                             start=True, stop=True)
            y2 = y2p.tile([8, g1 - g0], f32, tag="y2g")
            nc.scalar.activation(y2[:], y1, AF.Prelu,
                                 bias=cc('cab')[0:8, :], scale=cc('cas')[0:8, :],
                                 alpha=cc('caa')[0:8, :])
            ahgt = pss.tile([C, 512], f32, tag="small")
            ahg = ahgt[0:C, 0:g1 - g0]
            nc.tensor.matmul(ahg, crf('cawh_t', rows=8), y2[:],
                             start=True, stop=True)
            nc.scalar.activation(ah[:, g0:g1], ahg, AF.Sigmoid, bias=zcol,
                                 scale=1.0)

        def ah_gate(pre3, j, eng=None):
            r0, r1 = j * BH, (j + 1) * BH
            ah_b = ah[:, r0:r1].unsqueeze(2).broadcast_to((C, BH, HB))
            (eng or nc.gpsimd).tensor_tensor(pre3[:, r0:r1, :],
                                             pre3[:, r0:r1, :], ah_b,
                                             op=ALU.mult)

        def aw_gate(pre3, j, awX, eng=None):
            r0, r1 = j * BH, (j + 1) * BH
            aw_b = awX[:].unsqueeze(1).broadcast_to((C, BH, HB))
            (eng or nc.vector).tensor_tensor(pre3[:, r0:r1, :],
                                             pre3[:, r0:r1, :], aw_b,
                                             op=ALU.mult)

        def out_dma(side, j, eng):
            r0, r1 = j * BH, (j + 1) * BH
            pre3 = preL3 if side == 0 else preR3
            eng.dma_start(o3[:, side, r0:r1, :], pre3[:, r0:r1, :])

        def out_dma2(side, j, eng):
            r0, r1 = j * BH, (j + 2) * BH
            pre3 = preL3 if side == 0 else preR3
            eng.dma_start(o3[:, side, r0:r1, :], pre3[:, r0:r1, :])

        # ---- left phase ----
        for b in range(NB):
            do_block(0, b, preL3, yinL, xwacL3, first=(b == 0))
        aw_chain(xwacL3, xwL, awL)

        # ---- right phase; group CA + left-half drain overlapped ----
        for b in range(NB):
            do_block(1, b, preR3, yinR, xwacR3, first=(b == 0),
                     late=(b == NB - 1))
            if b in GROUPS:
                g0, g1 = GROUPS[b]
                ca_group(g0, g1)
                for j in range(g0 // BH, (g1 + BH - 1) // BH):
                    if b != NB - 1:
                        ah_gate(preR3, j)
                    ah_gate(preL3, j)
                    aw_gate(preL3, j, awL)
                    out_dma(0, j, nc.sync)

        # ---- tail: aw(R), right-half drain ----
        with tc.high_priority():
            aw_chain(xwacR3, xwR, awR, eng=nc.gpsimd)
        ah_gate(preR3, 15)

        def aw_gate4(j0, eng):
            r0, r1 = j0 * BH, (j0 + 4) * BH
            aw_b = awR[:].unsqueeze(1).broadcast_to((C, 4 * BH, HB))
            eng.tensor_tensor(preR3[:, r0:r1, :], preR3[:, r0:r1, :], aw_b,
                              op=ALU.mult)

        def out_dma4(j0, eng):
            r0, r1 = j0 * BH, (j0 + 4) * BH
            eng.dma_start(o3[:, 1, r0:r1, :], preR3[:, r0:r1, :])

        for j in range(NB):
            aw_gate(preR3, j, awR, eng=nc.gpsimd if j % 2 == 1 else nc.vector)
            if j % 2 == 1:
                out_dma2(1, j - 1, nc.sync if j % 4 == 1 else nc.scalar)

    nc.compile()
    return nc


def _get_graph(key, pairs, colf, colb, colq, ckf, ckb, ckq):
    if key not in _GRAPH_CACHE:
        _GRAPH_CACHE[key] = _build(pairs, colf, colb, colq, ckf, ckb, ckq)
    return _GRAPH_CACHE[key]


# ------------------------------------------------------------------ interface
def _run(inputs, trace=False):
    x = np.ascontiguousarray(np.asarray(inputs['x'], np.float32))
    assert x.shape == (B, C, H, W)
    (consts, colf, constb, colb, constq, colq, pairs, key) = _fold(inputs)
    nc = _get_graph(key, pairs, colf, colb, colq,
                    consts.shape[1], constb.shape[1], constq.shape[1])
    xb = x.astype(ml_dtypes.bfloat16).reshape(B, C, HW)
    xpad = np.zeros((B, C, HP, WP), ml_dtypes.float8_e4m3)
    xpad[:, :, PAD:PAD + H, PAD:PAD + W] = x.astype(ml_dtypes.float8_e4m3)
    in_maps = []
    for i in range(N_CORES):
        in_maps.append({'xb': xb[i].copy(),
                        'xpad': xpad[i].reshape(C, HP * WP).copy(),
                        'consts': consts, 'constb': constb, 'constq': constq})
    res = run_bass_kernel_spmd(nc, in_maps, list(range(N_CORES)), trace=trace)
    outs = []
    for i in range(N_CORES):
        o = res.results[i]['out'].astype(np.float32).reshape(C, 2, H, HB)
        outs.append(np.concatenate([o[:, 0], o[:, 1]], axis=2))
    return np.stack(outs, axis=0), res


def kernel(**inputs):
    out, _ = _run(inputs, trace=False)
    return out


# revision 24
# speedup vs baseline: 1.0076x; 1.0076x over previous
"""Trainium2 Bass kernel for nn_Axial_PFCU_Continuous (dense_cnn).

All linear terms ride the PE:
  z = W0 @ x  (bf16; W0 = GAMMA*(Wf~ diag(c0) + diag(cB0)))
      + 16 shift terms (mixer taps at +-4/8/16 on H and W, edge taps at +-1)
        packed as 8 fp8 DoubleRow matmuls, two terms per matmul: the rhs is a
        zero-padded fp8 image and the DoubleRow "t" dimension is given an
        arbitrary 2-D displacement stride, so each matmul contracts two
        different shifted windows of the same buffer (cost: 0.5 cyc/pixel
        per matmul, contraction-independent).
  pre = PReLU(z/16 + bz) on Act; coord-attention stats on DVE/Pool.

The image is processed in two column halves (left cols 0:64, then right),
so the left half's aw gating + output DMA overlap the right half's compute.

Sharding: pure data-parallel, 1 of 8 batch samples per NeuronCore.
"""
import sys
import math

sys.path.insert(0, '/opt/trn_rl_repo')

import numpy as np
import ml_dtypes
from contextlib import ExitStack

import concourse.bass as bass
import concourse.bacc as bacc
from concourse import mybir, tile
from concourse.bass_utils import run_bass_kernel_spmd
from concourse.ap import AP as APC

f32 = mybir.dt.float32
bf16 = mybir.dt.bfloat16
fp8 = mybir.dt.float8e4
ALU = mybir.AluOpType
AF = mybir.ActivationFunctionType
PM = mybir.MatmulPerfMode
AX = mybir.AxisListType

B, C, H, W = 8, 96, 128, 128
HW = H * W
PAD = 16
HP, WP = H + 2 * PAD, W + 2 * PAD      # padded fp8 image dims (160 x 160)
EPS = 1e-5
GAMMA = 16.0
HB = W // 2                            # half-width column split
BH = 8                                 # rows per block
NB = H // BH                           # 16 blocks per phase
N_CORES = 8
# ah groups (closed after the right-phase block that completes each row range)
GROUPS = {3: (0, 32), 7: (32, 64), 11: (64, 96), 12: (96, 104),
          13: (104, 112), 14: (112, 120), 15: (120, 128)}

_GRAPH_CACHE = {}


# ----------------------------------------------------------------- host folds
def _taps(w_taps, r):
    r = max(float(r), 1.0)
    K = w_taps.shape[1]
    d2w = {}
    for i in range(K):
        s = (i - K // 2) * r
        f = math.floor(s)
        frac = s - f
        for d, wt in ((int(f), 1.0 - frac), (int(f) + 1, frac)):
            if wt != 0.0:
                if d not in d2w:
                    d2w[d] = np.zeros(C, np.float64)
                d2w[d] = d2w[d] + wt * np.asarray(w_taps[:, i], np.float64)
    return {d: w for d, w in d2w.items() if abs(d) < H}


def _merge(a, b):
    out = dict(a)
    for d, w in b.items():
        out[d] = out.get(d, np.zeros(C, np.float64)) + w
    return out


class _Pack:
    def __init__(self, rows):
        self.rows = rows
        self.cols = {}
        self.parts = []
        self.pos = 0

    def put(self, name, arr):
        arr = np.asarray(arr, np.float64)
        if arr.ndim == 1:
            arr = arr[:, None]
        pad = np.zeros((self.rows, arr.shape[1]), np.float64)
        pad[:arr.shape[0], :] = arr
        self.cols[name] = (self.pos, arr.shape[1])
        self.parts.append(pad)
        self.pos += arr.shape[1]

    def done(self, dt):
        return np.concatenate(self.parts, axis=1).astype(dt)


def _fold(inp):
    g = lambda k: np.asarray(inp[k], np.float64)
    hA = _merge(_taps(g('wh_m'), float(np.asarray(inp['r_m']))),
                _taps(g('wh_l'), float(np.asarray(inp['r_l']))))
    wA = _merge(_taps(g('ww_m'), float(np.asarray(inp['r_m']))),
                _taps(g('ww_l'), float(np.asarray(inp['r_l']))))
    hA[0] = hA.get(0, np.zeros(C)) + 2.0    # identity terms of m+l
    wA.setdefault(0, np.zeros(C))
    c0 = hA[0] + wA[0]

    sf = g('bnf_g') / np.sqrt(g('bnf_v') + EPS)
    wf = g('w_fuse') * sf[:, None]            # (Cout, Cin) BN-folded
    bf_ = g('bnf_b') - g('bnf_m') * sf

    ds = g('dg_g') / np.sqrt(g('dg_v') + EPS)
    db = g('dg_b') - g('dg_m') * ds
    dg_wh, dg_ww = g('dg_wh'), g('dg_ww')
    ehm1, eh0, ehp1 = ds * dg_wh[:, 0], ds * (dg_wh[:, 1] + 1.0), ds * dg_wh[:, 2]
    ewm1, ew0, ewp1 = ds * dg_ww[:, 0], ds * dg_ww[:, 1], ds * dg_ww[:, 2]
    cB0 = eh0 + ew0
    bz = bf_ + db

    cs = g('ca_g') / np.sqrt(g('ca_v') + EPS)
    cb = g('ca_b') - g('ca_m') * cs

    # fp8 PE terms: (dr, dc, (Cout, Cin) matrix), all GAMMA-scaled
    terms = []
    for d in sorted(hA):
        if d != 0:
            assert abs(d) <= PAD, d
            terms.append((d, 0, GAMMA * wf * hA[d][None, :]))
    for d in sorted(wA):
        if d != 0:
            assert abs(d) <= PAD, d
            terms.append((0, d, GAMMA * wf * wA[d][None, :]))
    terms.append((-1, 0, np.diag(GAMMA * ehm1)))
    terms.append((1, 0, np.diag(GAMMA * ehp1)))
    terms.append((0, -1, np.diag(GAMMA * ewm1)))
    terms.append((0, 1, np.diag(GAMMA * ewp1)))
    if len(terms) % 2:
        terms.append((0, 0, np.zeros((C, C))))
    terms.sort(key=lambda t: t[0] * WP + t[1])

    pairs = []
    pkq = _Pack(C)
    for i in range(0, len(terms), 2):
        r0_, c0_, A0 = terms[i]
        r1_, c1_, A1 = terms[i + 1]
        s_t = (r1_ - r0_) * WP + (c1_ - c0_)
        assert s_t > 0, (terms[i][:2], terms[i + 1][:2])
        pairs.append(((r0_, c0_), s_t))
        pkq.put(f'P{i // 2}', np.concatenate([A0.T, A1.T], axis=1))
    constq = pkq.done(ml_dtypes.float8_e4m3)

    pkb = _Pack(C)
    W0 = GAMMA * (wf * c0[None, :] + np.diag(cB0))
    pkb.put('W0T', W0.T)
    pkb.put('caw1_tb', (g('ca_w1') / float(W)).T)     # (C, 8)
    pkb.put('caww_tb', g('ca_ww').T)                  # (8, C)
    constb = pkb.done(ml_dtypes.bfloat16)

    pkf = _Pack(C)
    pkf.put('bz', bz)
    pkf.put('act_a', g('act_a'))
    pkf.put('zero', np.zeros(C))
    pkf.put('caw1_t', (g('ca_w1') / float(W)).T)      # (C, 8) f32
    pkf.put('cas', cs)
    pkf.put('cab', cb)
    pkf.put('caa', g('ca_a'))
    pkf.put('cawh_t', g('ca_wh').T)                   # (8, C)
    consts = pkf.done(np.float32)

    key = (tuple(pairs), consts.shape[1], constb.shape[1], constq.shape[1])
    return consts, pkf.cols, constb, pkb.cols, constq, pkq.cols, pairs, key


# -------------------------------------------------------------- graph builder
def _build(pairs, colf, colb, colq, ckf, ckb, ckq):
    nc = bacc.Bacc()
    xb_p = nc.declare_dram_parameter("xb", (C, HW), bf16, isOutput=False)
    xp_p = nc.declare_dram_parameter("xpad", (C, HP * WP), fp8, isOutput=False)
    cf_p = nc.declare_dram_parameter("consts", (C, ckf), f32, isOutput=False)
    cb_p = nc.declare_dram_parameter("constb", (C, ckb), bf16, isOutput=False)
    cq_p = nc.declare_dram_parameter("constq", (C, ckq), fp8, isOutput=False)
    # out laid out as [C, side(2), H, HB]; host concatenates the halves
    o_p = nc.declare_dram_parameter("out", (C, HW), bf16, isOutput=True)

    with tile.TileContext(nc) as tc, ExitStack() as ctx:
        big = ctx.enter_context(tc.tile_pool(name="big", bufs=1))
        f1p = ctx.enter_context(tc.tile_pool(name="f1p", bufs=4))
        f2p = ctx.enter_context(tc.tile_pool(name="f2p", bufs=4))
        awf = ctx.enter_context(tc.tile_pool(name="awf", bufs=2))
        y2p = ctx.enter_context(tc.tile_pool(name="y2p", bufs=2))
        psq = ctx.enter_context(tc.tile_pool(name="psq", bufs=5, space="PSUM"))
        psw = ctx.enter_context(tc.tile_pool(name="psw", bufs=1, space="PSUM"))
        pss = ctx.enter_context(tc.tile_pool(name="pss", bufs=2, space="PSUM"))

        cst = big.tile([C, ckf], f32, tag="cst")
        cbt = big.tile([C, ckb], bf16, tag="cbt")
        cqt = big.tile([C, ckq], fp8, tag="cqt")

        def cc(name):
            p0, _ = colf[name]
            return cst[:, p0:p0 + 1]

        def crf(name, rows=C):
            p0, n = colf[name]
            return cst[0:rows, p0:p0 + n]

        def cbr(name, rows=C):
            p0, n = colb[name]
            return cbt[0:rows, p0:p0 + n]

        def cq(i):
            p0, n = colq[f'P{i}']
            return cqt[0:C, p0:p0 + n].rearrange("p (t m) -> p t m", t=2)

        xb_sb = big.tile([C, HW], bf16, tag="xb")
        xp_sb = big.tile([C, HP * WP], fp8, tag="xpad")
        preL = big.tile([C, H * HB], bf16, tag="preL")
        preR = big.tile([C, H * HB], bf16, tag="preR")
        scr = big.tile([C, 512], bf16, tag="scr")
        xwacL = big.tile([C, BH * HB], bf16, tag="xwacL")
        xwacR = big.tile([C, BH * HB], bf16, tag="xwacR")
        yinL = big.tile([C, H], f32, tag="yinL")
        yinR = big.tile([C, H], f32, tag="yinR")
        ying = big.tile([C, H], f32, tag="ying")
        ah = big.tile([C, H], bf16, tag="ah")
        awL = big.tile([C, HB], bf16, tag="awL")
        awR = big.tile([C, HB], bf16, tag="awR")
        xwL = big.tile([C, HB], bf16, tag="xwL")
        xwR = big.tile([C, HB], bf16, tag="xwR")

        xb3 = xb_sb[:].rearrange("p (h w) -> p h w", w=W)
        preL3 = preL[:].rearrange("p (h w) -> p h w", w=HB)
        preR3 = preR[:].rearrange("p (h w) -> p h w", w=HB)
        xwacL3 = xwacL[:].rearrange("p (h w) -> p h w", w=HB)
        xwacR3 = xwacR[:].rearrange("p (h w) -> p h w", w=HB)
        o3 = o_p[:].rearrange("p (s h w) -> p s h w", s=2, w=HB)
        xp_t = xp_sb[:].tensor
        zcol = cc('zero')

        def xpdma(eng, a, b):
            eng.dma_start(xp_sb[:, a * WP:b * WP], xp_p[:, a * WP:b * WP])

        def xbdma(eng, a, b):
            eng.dma_start(xb_sb[:, a * W:b * W], xb_p[:, a * W:b * W])

        # ---- input DMA schedule (first-needed first, 3 queues) ----
        xbdma(nc.scalar, 0, 8)
        nc.scalar.dma_start(cbt[:], cb_p[:])
        nc.scalar.dma_start(cqt[:], cq_p[:])
        nc.scalar.dma_start(cst[:], cf_p[:])
        xbdma(nc.scalar, 8, 16)
        xbdma(nc.scalar, 16, 32)
        xpdma(nc.sync, 0, 22)
        xpdma(nc.sync, 44, 66)
        xbdma(nc.sync, 32, 56)
        xbdma(nc.sync, 56, 88)
        xbdma(nc.sync, 88, 128)
        nc.gpsimd.memset(scr[:], 0.0)
        xpdma(nc.gpsimd, 22, 44)
        xpdma(nc.gpsimd, 66, 98)
        xpdma(nc.gpsimd, 98, 130)
        xpdma(nc.gpsimd, 130, 160)

        # ---- PE p-state warmup on zeroed scratch ----
        for i in range(4):
            pw = psw.tile([C, 512], f32, tag="warm")
            nc.tensor.matmul(pw[:], scr[:, 0:96], scr[:], start=True, stop=True)

        def qrhs(r0, cside, pair):
            (dr, dc), s_t = pair
            off = (r0 + PAD + dr) * WP + (PAD + dc + cside)
            return APC(xp_t, off, [[HP * WP, C], [s_t, 2], [WP, BH], [1, HB]])

        def do_block(side, b, pre3, yinX, xwac3, first=False, late=False):
            r0, r1 = b * BH, (b + 1) * BH
            cs_ = side * HB
            pk = psq.tile([C, BH, HB], f32, tag="pk")
            nc.tensor.matmul(pk[:], cbr('W0T'), xb3[:, r0:r1, cs_:cs_ + HB],
                             start=True, stop=False)
            for i, pr in enumerate(pairs):
                nc.tensor.matmul(pk[:], cq(i), qrhs(r0, cs_, pr),
                                 start=False, stop=(i == len(pairs) - 1),
                                 perf_mode=PM.DoubleRow)
            nc.scalar.activation(pre3[:, r0:r1, :], pk[:], AF.Prelu,
                                 bias=cc('bz'), scale=1.0 / GAMMA,
                                 alpha=cc('act_a'))
            if first:
                nc.vector.tensor_copy(xwac3, pre3[:, r0:r1, :])
            else:
                xw_eng = nc.gpsimd if late else nc.vector
                xw_eng.tensor_tensor(xwac3, xwac3, pre3[:, r0:r1, :],
                                     op=ALU.add)
            f1 = f1p.tile([C, BH * (HB // 2)], bf16, tag="f1")
            f13 = f1[:].rearrange("p (h w) -> p h w", w=HB // 2)
            nc.gpsimd.tensor_tensor(f13, pre3[:, r0:r1, 0:HB // 2],
                                    pre3[:, r0:r1, HB // 2:HB], op=ALU.add)
            f2 = f2p.tile([C, BH * (HB // 4)], bf16, tag="f2")
            f23 = f2[:].rearrange("p (h w) -> p h w", w=HB // 4)
            nc.gpsimd.tensor_tensor(f23, f13[:, :, 0:HB // 4],
                                    f13[:, :, HB // 4:HB // 2], op=ALU.add)
            nc.vector.tensor_reduce(yinX[:, r0:r1], f23, axis=AX.X, op=ALU.add)

        def aw_chain(xwac3, xwX, awX, eng=None):
            eng = eng or nc.vector
            t1 = awf.tile([C, 4 * HB], bf16, tag="awt1")
            t13 = t1[:].rearrange("p (h w) -> p h w", w=HB)
            eng.tensor_tensor(t13, xwac3[:, 0:4, :], xwac3[:, 4:8, :],
                              op=ALU.add)
            t2 = awf.tile([C, 2 * HB], bf16, tag="awt2")
            t23 = t2[:].rearrange("p (h w) -> p h w", w=HB)
            eng.tensor_tensor(t23, t13[:, 0:2, :], t13[:, 2:4, :],
                              op=ALU.add)
            eng.tensor_tensor(xwX[:], t2[:, 0:HB], t2[:, HB:2 * HB],
                              op=ALU.add)
            y1wt = pss.tile([C, 512], f32, tag="small")
            y1w = y1wt[0:8, 0:HB]
            nc.tensor.matmul(y1w, cbr('caw1_tb'), xwX[:],
                             start=True, stop=True)
            y2w = y2p.tile([8, HB], bf16, tag="y2w")
            nc.scalar.activation(y2w[:], y1w, AF.Prelu,
                                 bias=cc('cab')[0:8, :], scale=cc('cas')[0:8, :],
                                 alpha=cc('caa')[0:8, :])
            awpt = pss.tile([C, 512], f32, tag="small")
            awp = awpt[0:C, 0:HB]
            nc.tensor.matmul(awp, cbr('caww_tb', rows=8), y2w[:],
                             start=True, stop=True)
            nc.scalar.activation(awX[:], awp, AF.Sigmoid, bias=zcol,
                                 scale=1.0)

        def ca_group(g0, g1):
            nc.vector.tensor_tensor(ying[:, g0:g1], yinL[:, g0:g1],
                                    yinR[:, g0:g1], op=ALU.add)
            y1t = pss.tile([C, 512], f32, tag="small")
            y1 = y1t[0:8, 0:g1 - g0]
            nc.tensor.matmul(y1, crf('caw1_t'), ying[:, g0:g1],
                             start=True, stop=True)
            y2 = y2p.tile([8, g1 - g0], f32, tag="y2g")
            nc.scalar.activation(y2[:], y1, AF.Prelu,
                                 bias=cc('cab')[0:8, :], scale=cc('cas')[0:8, :],
                                 alpha=cc('caa')[0:8, :])
            ahgt = pss.tile([C, 512], f32, tag="small")
            ahg = ahgt[0:C, 0:g1 - g0]
            nc.tensor.matmul(ahg, crf('cawh_t', rows=8), y2[:],
                             start=True, stop=True)
            nc.scalar.activation(ah[:, g0:g1], ahg, AF.Sigmoid, bias=zcol,
                                 scale=1.0)

        def ah_gate(pre3, j, eng=None):
            r0, r1 = j * BH, (j + 1) * BH
            ah_b = ah[:, r0:r1].unsqueeze(2).broadcast_to((C, BH, HB))
            (eng or nc.gpsimd).tensor_tensor(pre3[:, r0:r1, :],
                                             pre3[:, r0:r1, :], ah_b,
                                             op=ALU.mult)

        def aw_gate(pre3, j, awX, eng=None):
            r0, r1 = j * BH, (j + 1) * BH
            aw_b = awX[:].unsqueeze(1).broadcast_to((C, BH, HB))
            (eng or nc.vector).tensor_tensor(pre3[:, r0:r1, :],
                                             pre3[:, r0:r1, :], aw_b,
                                             op=ALU.mult)

        def out_dma(side, j, eng):
            r0, r1 = j * BH, (j + 1) * BH
            pre3 = preL3 if side == 0 else preR3
            eng.dma_start(o3[:, side, r0:r1, :], pre3[:, r0:r1, :])

        def out_dma2(side, j, eng):
            r0, r1 = j * BH, (j + 2) * BH
            pre3 = preL3 if side == 0 else preR3
            eng.dma_start(o3[:, side, r0:r1, :], pre3[:, r0:r1, :])

        def gate32(pre3, r0, vec, eng):
            v_b = vec[:].unsqueeze(1).broadcast_to((C, 32, HB))
            eng.tensor_tensor(pre3[:, r0:r0 + 32, :], pre3[:, r0:r0 + 32, :],
                              v_b, op=ALU.mult)

        def ah32(pre3, r0, eng):
            ah_b = ah[:, r0:r0 + 32].unsqueeze(2).broadcast_to((C, 32, HB))
            eng.tensor_tensor(pre3[:, r0:r0 + 32, :], pre3[:, r0:r0 + 32, :],
                              ah_b, op=ALU.mult)

        def dma32(side, r0, eng):
            pre3 = preL3 if side == 0 else preR3
            eng.dma_start(o3[:, side, r0:r0 + 32, :], pre3[:, r0:r0 + 32, :])

        # ---- left phase ----
        for b in range(NB):
            do_block(0, b, preL3, yinL, xwacL3, first=(b == 0))
        aw_chain(xwacL3, xwL, awL)

        # ---- right phase; group CA + early-rows drain overlapped ----
        for b in range(NB):
            do_block(1, b, preR3, yinR, xwacR3, first=(b == 0),
                     late=(b == NB - 1))
            if b in GROUPS:
                g0, g1 = GROUPS[b]
                ca_group(g0, g1)
                if b <= 11:
                    # early groups: both-half ah gating + left drain now
                    for j in range(g0 // BH, (g1 + BH - 1) // BH):
                        ah_gate(preR3, j)
                        ah_gate(preL3, j)
                        aw_gate(preL3, j, awL)
                        out_dma(0, j, nc.sync)

        # ---- tail: critical chains first, then bulk 32-row gates ----
        with tc.high_priority():
            aw_chain(xwacR3, xwR, awR, eng=nc.gpsimd)
        ah32(preR3, 96, nc.gpsimd)
        ah32(preL3, 96, nc.gpsimd)
        gate32(preL3, 96, awL, nc.vector)
        dma32(0, 96, nc.sync)
        gate32(preR3, 0, awR, nc.vector)
        dma32(1, 0, nc.sync)
        gate32(preR3, 32, awR, nc.vector)
        dma32(1, 32, nc.scalar)
        gate32(preR3, 64, awR, nc.vector)
        dma32(1, 64, nc.sync)
        gate32(preR3, 96, awR, nc.gpsimd)
        dma32(1, 96, nc.scalar)

    nc.compile()
    return nc


def _get_graph(key, pairs, colf, colb, colq, ckf, ckb, ckq):
    if key not in _GRAPH_CACHE:
        _GRAPH_CACHE[key] = _build(pairs, colf, colb, colq, ckf, ckb, ckq)
    return _GRAPH_CACHE[key]


# ------------------------------------------------------------------ interface
def _run(inputs, trace=False):
    x = np.ascontiguousarray(np.asarray(inputs['x'], np.float32))
    assert x.shape == (B, C, H, W)
    (consts, colf, constb, colb, constq, colq, pairs, key) = _fold(inputs)
    nc = _get_graph(key, pairs, colf, colb, colq,
                    consts.shape[1], constb.shape[1], constq.shape[1])
    xb = x.astype(ml_dtypes.bfloat16).reshape(B, C, HW)
    xpad = np.zeros((B, C, HP, WP), ml_dtypes.float8_e4m3)
    xpad[:, :, PAD:PAD + H, PAD:PAD + W] = x.astype(ml_dtypes.float8_e4m3)
    in_maps = []
    for i in range(N_CORES):
        in_maps.append({'xb': xb[i].copy(),
                        'xpad': xpad[i].reshape(C, HP * WP).copy(),
                        'consts': consts, 'constb': constb, 'constq': constq})
    res = run_bass_kernel_spmd(nc, in_maps, list(range(N_CORES)), trace=trace)
    outs = []
    for i in range(N_CORES):
        o = res.results[i]['out'].astype(np.float32).reshape(C, 2, H, HB)
        outs.append(np.concatenate([o[:, 0], o[:, 1]], axis=2))
    return np.stack(outs, axis=0), res


def kernel(**inputs):
    out, _ = _run(inputs, trace=False)
    return out
